# revision 8
# baseline (speedup 1.0000x reference)
"""Bass/Trainium2 kernel for nn_BesselEdgeLengthEncoding (segment_reduce).

Sharding: by destination-node groups (the sharding hint). Edges arrive
sorted by dst with exactly DEG=32 edges per node, so core c owns nodes
[c*6250, (c+1)*6250) == contiguous edge slice [c*200000, (c+1)*200000).
All compute is per-node independent -> 8 fully independent SPMD shards,
no collectives.

On-chip layout: the core's 6250 nodes are padded to 6272 = 128*49.
Partition p holds nodes p*49 + blk (blk in 0..48); every per-edge tensor
is an SBUF tile [128, 49, 32] (6272 B/partition) and stays resident for
the whole kernel.  DMAs are therefore fully contiguous per partition.

Math notes (matching reference.py):
  norm_length == edge_length/6 exactly, so it is recomputed on-chip.
  sigmoid(10*D) = 0.5 + 0.5*tanh(5*D)  -> ranks via tanh, which shares
  the ACT table set "exp_and_others" with exp/square (1 table load);
  sin for the bessel phase is the only other set (trig).
  u^50 is computed by repeated squaring (avoids the Ln table set).
  bessel*env: emb[e,k] = sin(pi*w_k*d/6) * env6 * 6/(pi*d), and
  env6*keep == env6 exactly (poly6 <= 0 wherever keep is 0).
"""

import math
from contextlib import ExitStack

import numpy as np

import concourse.bacc as bacc
import concourse.bass as bass
import concourse.tile as tile
from concourse import mybir
from concourse.bass_utils import run_bass_kernel_spmd

AF = mybir.ActivationFunctionType
OP = mybir.AluOpType
AX = mybir.AxisListType
F32 = mybir.dt.float32
U8 = mybir.dt.uint8

N = 50000
DEG = 32
NB = 8
HC = 6.0
N_CORES = 8
NODES_PER_CORE = N // N_CORES  # 6250
P = 128
BLKS = (NODES_PER_CORE + P - 1) // P  # 49
PAD_NODES = P * BLKS  # 6272
CHUNK = 7  # pairwise chunk size in blocks (divides BLKS)
PAD_D = 3.0  # edge length filled into padding nodes (any positive value)
LOG_C = math.log(10000.0) - math.log(4.0) - 0.5 * math.log(2.0 * math.pi)
PI = math.pi


def _emit(tc, blks, chunk, d_in, emb_out, cut_out, keep_out, tag=""):
    nc = tc.nc
    shp3 = [P, blks, DEG]
    shp4 = [P, blks, DEG, NB]

    with ExitStack() as ctx:
        res = ctx.enter_context(tc.tile_pool(name=f"res{tag}", bufs=1))
        pw = ctx.enter_context(tc.tile_pool(name=f"pw{tag}", bufs=2))

        # ---- load edge lengths, fully resident ----
        d = res.tile(shp3, F32)
        nc.sync.dma_start(out=d, in_=d_in.rearrange("p (c e) -> p c e", e=DEG))

        bias_m3 = res.tile([P, 1], F32)
        nc.vector.memset(bias_m3, -3.0)
        bias_lc = res.tile([P, 1], F32)
        nc.vector.memset(bias_lc, LOG_C)
        bias_mpi = res.tile([P, 1], F32)
        nc.vector.memset(bias_mpi, -PI)

        # ---- phase A: poly50 envelope  penv = max(1 + u^50*(-1326 + 2600u - 1275u^2), 0)
        u = res.tile(shp3, F32)
        nc.vector.tensor_scalar(u, d, 1.0 / HC, None, op0=OP.mult)
        tA = res.tile(shp3, F32)  # u^2 (kept)
        tB = res.tile(shp3, F32)
        tC = res.tile(shp3, F32)
        nc.scalar.activation(tA, u, AF.Square)          # u^2
        nc.scalar.activation(tB, tA, AF.Square)         # u^4
        nc.scalar.activation(tB, tB, AF.Square)         # u^8
        nc.scalar.activation(tB, tB, AF.Square)         # u^16
        nc.scalar.activation(tC, tB, AF.Square)         # u^32
        nc.vector.tensor_mul(tC, tC, tB)                # u^48
        nc.vector.tensor_mul(tC, tC, tA)                # u^50
        nc.vector.tensor_scalar(tB, u, -1275.0, 2600.0, op0=OP.mult, op1=OP.add)
        nc.vector.tensor_mul(tB, tB, u)                 # 2600u - 1275u^2
        nc.vector.tensor_scalar(tB, tB, -1326.0, None, op0=OP.add)
        penv = res.tile(shp3, F32)
        nc.vector.tensor_mul(penv, tC, tB)              # u^50 * (...)
        nc.vector.tensor_scalar(penv, penv, 1.0, 0.0, op0=OP.add, op1=OP.max)

        # P_sum[p, c] = sum_b penv
        psum = res.tile([P, blks], F32)
        nc.vector.reduce_sum(psum, penv, axis=AX.X)

        # ---- pairwise:  rk[p,c,a] = sum_b tanh(5*(d_a - d_b)) * penv_b
        rk = res.tile(shp3, F32)
        for c0 in range(0, blks, chunk):
            sl = slice(c0, c0 + chunk)
            dT = pw.tile([P, chunk, DEG, DEG], F32, tag="dT")
            nc.vector.tensor_tensor(
                dT,
                d[:, sl, :].unsqueeze(3).broadcast_to([P, chunk, DEG, DEG]),
                d[:, sl, :].unsqueeze(2).broadcast_to([P, chunk, DEG, DEG]),
                op=OP.subtract,
            )
            nc.scalar.activation(dT, dT, AF.Tanh, scale=5.0)
            nc.vector.tensor_mul(
                dT, dT, penv[:, sl, :].unsqueeze(2).broadcast_to([P, chunk, DEG, DEG])
            )
            nc.vector.reduce_sum(rk[:, sl, :], dT, axis=AX.X)

        # ranks*2 = psum + rk - penv ; gaussian rank weights
        nc.vector.tensor_sub(rk, rk, penv)
        nc.vector.tensor_add(rk, rk, psum.unsqueeze(2).broadcast_to(shp3))
        # ((z/2 - 12)/4)^2 = (z/8 - 3)^2
        nc.scalar.activation(rk, rk, AF.Square, bias=bias_m3, scale=0.125)
        nc.scalar.activation(rk, rk, AF.Exp, bias=bias_lc, scale=-0.5)
        nc.vector.tensor_scalar(rk, rk, 1e-6, None, op0=OP.add)
        nc.vector.tensor_mul(rk, rk, penv)              # rank_weights

        # ---- cutoffs ----
        wd = tB
        nc.vector.tensor_mul(wd, rk, d)
        dsum = res.tile([P, blks], F32)
        wsum = res.tile([P, blks], F32)
        nc.vector.reduce_sum(dsum, wd, axis=AX.X)
        nc.vector.reduce_sum(wsum, rk, axis=AX.X)
        nc.vector.tensor_scalar(dsum, dsum, HC * 1e-6, None, op0=OP.add)
        nc.vector.tensor_scalar(wsum, wsum, 1e-6, None, op0=OP.add)
        iw = res.tile([P, blks], F32)
        nc.vector.reciprocal(iw, wsum)
        cut = res.tile([P, blks], F32)
        nc.vector.tensor_mul(cut, dsum, iw)

        cutb = res.tile(shp3, F32)
        nc.vector.tensor_copy(cutb, cut.unsqueeze(2).broadcast_to(shp3))
        keep = res.tile(shp3, U8)
        nc.vector.tensor_tensor(keep, d, cutb, op=OP.is_lt)
        nc.sync.dma_start(out=cut_out, in_=cutb.rearrange("p c e -> p (c e)"))
        nc.sync.dma_start(out=keep_out, in_=keep.rearrange("p c e -> p (c e)"))

        # ---- phase C: env6 + bessel ----
        ic = res.tile([P, blks], F32)
        nc.vector.reciprocal(ic, cut)
        v = u  # reuse
        nc.vector.tensor_mul(v, d, ic.unsqueeze(2).broadcast_to(shp3))
        # env = max(1 + v^6*(-28 + 48v - 21v^2), 0)
        nc.vector.tensor_scalar(tC, v, -21.0, 48.0, op0=OP.mult, op1=OP.add)
        nc.vector.tensor_mul(tC, tC, v)
        nc.vector.tensor_scalar(tC, tC, -28.0, None, op0=OP.add)
        nc.scalar.activation(tA, v, AF.Square)          # v^2
        nc.vector.tensor_mul(tA, tA, v)                 # v^3
        nc.vector.tensor_mul(tA, tA, tA)                # v^6
        nc.vector.tensor_mul(tC, tC, tA)
        nc.vector.tensor_scalar(tC, tC, 1.0, 0.0, op0=OP.add, op1=OP.max)  # env6

        invd = tB
        nc.vector.reciprocal_approx_accurate(
            out=invd.rearrange("p c e -> p (c e)"),
            in_=d.rearrange("p c e -> p (c e)"),
            scratch=tA.rearrange("p c e -> p (c e)"),
        )
        h = tC
        nc.vector.tensor_mul(h, h, invd)
        nc.vector.tensor_scalar(h, h, HC / PI, None, op0=OP.mult)  # env*6/(pi*d)

        # sin(pi*x_k), x_k = d*(k+1)/6: t = d*(k+1)/12, n = rint(t) via the
        # (t+1.5*2^23)-1.5*2^23 trick, f = t-n in [-1/2,1/2], sin(2*pi*f).
        RC = 1.5 * 2.0**23
        embs = res.tile(shp4, F32)
        for k in range(NB):
            t = pw.tile(shp3, F32, tag="t")
            nc.vector.tensor_scalar(t, d, (k + 1) / (2.0 * HC), None, op0=OP.mult)
            n = pw.tile(shp3, F32, tag="n")
            nc.vector.tensor_scalar(n, t, RC, RC, op0=OP.add, op1=OP.subtract)
            nc.vector.tensor_sub(t, t, n)
            nc.scalar.activation(embs[:, :, :, k], t, AF.Sin, scale=2.0 * PI)
        nc.vector.tensor_mul(embs, embs, h.unsqueeze(3).broadcast_to(shp4))
        nc.sync.dma_start(
            out=emb_out, in_=embs.rearrange("p c e k -> p (c e k)")
        )


def build(blks=BLKS, chunk=CHUNK, repeats=1):
    ne = blks * DEG
    nc = bacc.Bacc(
        "TRN2",
        target_bir_lowering=False,
        debug=False,
        enable_asserts=False,
        num_devices=N_CORES,
    )
    d_in = nc.dram_tensor("dlen", [P, ne], F32, kind="ExternalInput").ap()
    emb_out = nc.dram_tensor("emb", [P, ne * NB], F32, kind="ExternalOutput").ap()
    cut_out = nc.dram_tensor("cut", [P, ne], F32, kind="ExternalOutput").ap()
    keep_out = nc.dram_tensor("keep", [P, ne], U8, kind="ExternalOutput").ap()
    with tile.TileContext(nc) as tc:
        for r in range(repeats):
            _emit(tc, blks, chunk, d_in, emb_out, cut_out, keep_out, tag=str(r))
    nc.compile()
    return nc


_NC_CACHE = None


def _get_nc():
    global _NC_CACHE
    if _NC_CACHE is None:
        _NC_CACHE = build()
    return _NC_CACHE


def _shard_inputs(edge_length):
    """Full [E] edge lengths -> per-core [128, BLKS*DEG] padded layouts."""
    el = np.ascontiguousarray(np.asarray(edge_length, dtype=np.float32)).reshape(
        N_CORES, NODES_PER_CORE, DEG
    )
    maps = []
    for c in range(N_CORES):
        dpad = np.full((PAD_NODES, DEG), PAD_D, dtype=np.float32)
        dpad[:NODES_PER_CORE] = el[c]
        maps.append({"dlen": dpad.reshape(P, BLKS * DEG)})
    return maps


def _unshard(results):
    embs, cuts, keeps = [], [], []
    for r in results:
        emb = r["emb"].reshape(PAD_NODES, DEG, NB)[:NODES_PER_CORE]
        embs.append(emb.reshape(NODES_PER_CORE * DEG, NB))
        cut = r["cut"].reshape(PAD_NODES, DEG)[:NODES_PER_CORE]
        cuts.append(cut.reshape(-1))
        kp = r["keep"].reshape(PAD_NODES, DEG)[:NODES_PER_CORE]
        keeps.append(kp.reshape(-1))
    edge_emb = np.concatenate(embs, axis=0).astype(np.float32)
    cutoffs_edge = np.concatenate(cuts, axis=0).astype(np.float32)
    keep = np.concatenate(keeps, axis=0).astype(bool)
    return edge_emb, cutoffs_edge, keep


def run_on_hw(edge_length, **spmd_kwargs):
    nc = _get_nc()
    in_maps = _shard_inputs(edge_length)
    out = run_bass_kernel_spmd(nc, in_maps, list(range(N_CORES)), **spmd_kwargs)
    return out


def kernel(norm_length=None, edge_length=None, bessel_weights=None,
           edge_src=None, edge_dst=None, **_unused):
    """Full (unsharded) inputs -> full (edge_emb, cutoffs_edge, keep)."""
    assert edge_length is not None
    out = run_on_hw(edge_length)
    return _unshard(out.results)


# revision 23
# speedup vs baseline: 29.2486x; 29.2486x over previous
"""Bass/Trainium2 kernel for nn_BesselEdgeLengthEncoding (segment_reduce).

Sharding: by destination-node groups (the sharding hint). Edges arrive
sorted by dst with exactly DEG=32 edges per node, so core c owns nodes
[c*6250, (c+1)*6250) == contiguous edge slice [c*200000, (c+1)*200000).
All compute is per-node independent -> 8 fully independent SPMD shards,
no collectives.

On-chip layout: the core's 6250 nodes are padded to 6272 = 128*49.
Partition p holds nodes p*49 + blk (blk in 0..48); every per-edge tensor
is an SBUF tile [128, 49, 32] (6272 B/partition) and stays resident for
the whole kernel.  DMAs are therefore fully contiguous per partition.

Math notes (matching reference.py):
  norm_length == edge_length/6 exactly, so it is recomputed on-chip.
  sigmoid(10*D) = 0.5 + 0.5*tanh(5*D)  -> ranks via tanh, which shares
  the ACT table set "exp_and_others" with exp/square (1 table load);
  sin for the bessel phase is the only other set (trig).
  u^50 is computed by repeated squaring (avoids the Ln table set).
  bessel*env: emb[e,k] = sin(pi*w_k*d/6) * env6 * 6/(pi*d), and
  env6*keep == env6 exactly (poly6 <= 0 wherever keep is 0).
  sin args are range-reduced to [-pi, pi] (HW spline range) with the
  rint trick n = (t + 1.5*2^23) - 1.5*2^23.

Engine assignment: DVE does all elementwise/reduce work, ACT does
tanh/exp/squares/sin.  (GPSIMD offload was measured 3x slower: its Q7
software loop degrades on broadcast access patterns and it shares an
SBUF port with the DVE, which runs 2-port instructions continuously.)
"""

import math
import os
import tempfile
from contextlib import ExitStack

import numpy as np

# The neuron persistent compile cache keys on the XLA module fingerprint,
# which does NOT include the bass program embedded in the custom call's
# backend_config -- two different bass kernels with the same I/O signature
# collide.  Use a fresh cache dir so a stale NEFF can never be served.
os.environ["NEURON_COMPILE_CACHE_URL"] = tempfile.mkdtemp(prefix="neuron-cache-")

import concourse.bacc as bacc
import concourse.bass as bass
import concourse.tile as tile
from concourse import mybir
from concourse.bass_utils import run_bass_kernel_spmd

AF = mybir.ActivationFunctionType
OP = mybir.AluOpType
AX = mybir.AxisListType
F32 = mybir.dt.float32
U8 = mybir.dt.uint8

N = 50000
DEG = 32
NB = 8
HC = 6.0
N_CORES = 8
NODES_PER_CORE = N // N_CORES  # 6250
P = 128
BLKS = (NODES_PER_CORE + P - 1) // P  # 49
PAD_NODES = P * BLKS  # 6272
CHUNK = 7  # pairwise chunk size in blocks (divides BLKS)
PAD_D = 3.0  # edge length filled into padding nodes (any positive value)
LOG_C = math.log(10000.0) - math.log(4.0) - 0.5 * math.log(2.0 * math.pi)
PI = math.pi


def _emit(tc, blks, chunk, d_in, emb_out, cut_out, keep_out, tag=""):
    nc = tc.nc
    shp3 = [P, blks, DEG]
    shp4 = [P, blks, DEG, NB]
    ve = nc.vector

    with ExitStack() as ctx:
        res = ctx.enter_context(tc.tile_pool(name=f"res{tag}", bufs=1))
        pw = ctx.enter_context(tc.tile_pool(name=f"pw{tag}", bufs=2))

        # ---- load edge lengths, fully resident ----
        d = res.tile(shp3, F32)
        nc.sync.dma_start(out=d, in_=d_in.rearrange("p (c e) -> p c e", e=DEG))

        bias_m3 = res.tile([P, 1], F32)
        ve.memset(bias_m3, -3.0)
        bias_lc = res.tile([P, 1], F32)
        ve.memset(bias_lc, LOG_C)

        # ---- phase A: poly50 envelope  penv = max(1 + u^50*(-1326 + 2600u - 1275u^2), 0)
        u = res.tile(shp3, F32)
        ve.tensor_scalar(u, d, 1.0 / HC, None, op0=OP.mult)
        tA = res.tile(shp3, F32)  # u^2 (kept)
        tB = res.tile(shp3, F32)
        tC = res.tile(shp3, F32)
        nc.scalar.activation(tA, u, AF.Square)          # u^2
        nc.scalar.activation(tB, tA, AF.Square)         # u^4
        nc.scalar.activation(tB, tB, AF.Square)         # u^8
        nc.scalar.activation(tB, tB, AF.Square)         # u^16
        nc.scalar.activation(tC, tB, AF.Square)         # u^32
        ve.tensor_mul(tC, tC, tB)                       # u^48
        ve.tensor_mul(tC, tC, tA)                       # u^50
        ve.tensor_scalar(tB, u, -1275.0, 2600.0, op0=OP.mult, op1=OP.add)
        ve.tensor_mul(tB, tB, u)                        # 2600u - 1275u^2
        ve.tensor_scalar(tB, tB, -1326.0, None, op0=OP.add)
        penv = res.tile(shp3, F32)
        ve.tensor_mul(penv, tC, tB)                     # u^50 * (...)
        ve.tensor_scalar(penv, penv, 1.0, 0.0, op0=OP.add, op1=OP.max)

        # P_sum[p, c] = sum_b penv
        psum = res.tile([P, blks], F32)
        ve.reduce_sum(psum, penv, axis=AX.X)

        # ---- pairwise:  rk[p,c,a] = sum_b tanh(5*(d_a - d_b)) * penv_b
        rk = res.tile(shp3, F32)
        for c0 in range(0, blks, chunk):
            cn = min(chunk, blks - c0)
            sl = slice(c0, c0 + cn)
            dT = pw.tile([P, chunk, DEG, DEG], F32, tag="dT")
            w4 = [P, cn, DEG, DEG]
            ve.tensor_tensor(
                dT[:, :cn],
                d[:, sl, :].unsqueeze(3).broadcast_to(w4),
                d[:, sl, :].unsqueeze(2).broadcast_to(w4),
                op=OP.subtract,
            )
            nc.scalar.activation(dT[:, :cn], dT[:, :cn], AF.Tanh, scale=5.0)
            pb = penv[:, sl, :].unsqueeze(2).broadcast_to(w4)
            ve.tensor_mul(dT[:, :cn], dT[:, :cn], pb)
            ve.reduce_sum(rk[:, sl, :], dT[:, :cn], axis=AX.X)

        # ranks*2 = psum + rk - penv ; gaussian rank weights
        ve.tensor_sub(rk, rk, penv)
        ve.tensor_add(rk, rk, psum.unsqueeze(2).broadcast_to(shp3))
        # ((z/2 - 12)/4)^2 = (z/8 - 3)^2
        nc.scalar.activation(rk, rk, AF.Square, bias=bias_m3, scale=0.125)
        nc.scalar.activation(rk, rk, AF.Exp, bias=bias_lc, scale=-0.5)
        ve.tensor_scalar(rk, rk, 1e-6, None, op0=OP.add)
        ve.tensor_mul(rk, rk, penv)                     # rank_weights

        # ---- cutoffs ----
        wd = tB
        ve.tensor_mul(wd, rk, d)
        dsum = res.tile([P, blks], F32)
        wsum = res.tile([P, blks], F32)
        ve.reduce_sum(dsum, wd, axis=AX.X)
        ve.reduce_sum(wsum, rk, axis=AX.X)
        ve.tensor_scalar(dsum, dsum, HC * 1e-6, None, op0=OP.add)
        ve.tensor_scalar(wsum, wsum, 1e-6, None, op0=OP.add)
        iw = res.tile([P, blks], F32)
        ve.reciprocal(iw, wsum)
        cut = res.tile([P, blks], F32)
        ve.tensor_mul(cut, dsum, iw)

        cutb = res.tile(shp3, F32)
        ve.tensor_copy(cutb, cut.unsqueeze(2).broadcast_to(shp3))
        keep = res.tile(shp3, U8)
        ve.tensor_tensor(keep, d, cutb, op=OP.is_lt)
        nc.sync.dma_start(out=cut_out, in_=cutb.rearrange("p c e -> p (c e)"))
        nc.sync.dma_start(out=keep_out, in_=keep.rearrange("p c e -> p (c e)"))

        # ---- phase C: env6 + bessel ----
        ic = res.tile([P, blks], F32)
        ve.reciprocal(ic, cut)
        v = u  # reuse
        ve.tensor_mul(v, d, ic.unsqueeze(2).broadcast_to(shp3))
        # env = max(1 + v^6*(-28 + 48v - 21v^2), 0)
        ve.tensor_scalar(tC, v, -21.0, 48.0, op0=OP.mult, op1=OP.add)
        ve.tensor_mul(tC, tC, v)
        ve.tensor_scalar(tC, tC, -28.0, None, op0=OP.add)
        nc.scalar.activation(tA, v, AF.Square)          # v^2
        ve.tensor_mul(tA, tA, v)                        # v^3
        nc.scalar.activation(tA, tA, AF.Square)         # v^6
        ve.tensor_mul(tC, tC, tA)
        ve.tensor_scalar(tC, tC, 1.0, 0.0, op0=OP.add, op1=OP.max)  # env6

        invd = tB
        ve.reciprocal_approx_accurate(
            out=invd.rearrange("p c e -> p (c e)"),
            in_=d.rearrange("p c e -> p (c e)"),
            scratch=tA.rearrange("p c e -> p (c e)"),
        )
        h = tC
        ve.tensor_mul(h, h, invd)
        ve.tensor_scalar(h, h, HC / PI, None, op0=OP.mult)  # env*6/(pi*d)

        # sin(pi*x_k), x_k = d*(k+1)/6: t = d*(k+1)/12, n = rint(t) via the
        # (t+1.5*2^23)-1.5*2^23 trick, f = t-n in [-1/2,1/2], sin(2*pi*f).
        RC = 1.5 * 2.0**23
        embs = res.tile(shp4, F32)
        for k in range(NB):
            t = pw.tile(shp3, F32, tag="t")
            ve.tensor_scalar(t, d, (k + 1) / (2.0 * HC), None, op0=OP.mult)
            n = pw.tile(shp3, F32, tag="n")
            ve.tensor_scalar(n, t, RC, RC, op0=OP.add, op1=OP.subtract)
            ve.tensor_sub(t, t, n)
            nc.scalar.activation(embs[:, :, :, k], t, AF.Sin, scale=2.0 * PI)
        ve.tensor_mul(embs, embs, h.unsqueeze(3).broadcast_to(shp4))
        nc.sync.dma_start(
            out=emb_out, in_=embs.rearrange("p c e k -> p (c e k)")
        )


def build(blks=BLKS, chunk=CHUNK, repeats=1):
    ne = blks * DEG
    nc = bacc.Bacc(
        "TRN2",
        target_bir_lowering=False,
        debug=False,
        enable_asserts=False,
        num_devices=N_CORES,
    )
    d_in = nc.dram_tensor("dlen", [P, ne], F32, kind="ExternalInput").ap()
    emb_out = nc.dram_tensor("emb", [P, ne * NB], F32, kind="ExternalOutput").ap()
    cut_out = nc.dram_tensor("cut", [P, ne], F32, kind="ExternalOutput").ap()
    keep_out = nc.dram_tensor("keep", [P, ne], U8, kind="ExternalOutput").ap()
    with tile.TileContext(nc) as tc:
        if repeats == 1:
            _emit(tc, blks, chunk, d_in, emb_out, cut_out, keep_out)
        else:
            with tc.For_i(0, repeats, 1):
                _emit(tc, blks, chunk, d_in, emb_out, cut_out, keep_out)
    nc.compile()
    return nc


_NC_CACHE = None


def _get_nc():
    global _NC_CACHE
    if _NC_CACHE is None:
        _NC_CACHE = build()
    return _NC_CACHE


def _shard_inputs(edge_length):
    """Full [E] edge lengths -> per-core [128, BLKS*DEG] padded layouts."""
    el = np.ascontiguousarray(np.asarray(edge_length, dtype=np.float32)).reshape(
        N_CORES, NODES_PER_CORE, DEG
    )
    maps = []
    for c in range(N_CORES):
        dpad = np.full((PAD_NODES, DEG), PAD_D, dtype=np.float32)
        dpad[:NODES_PER_CORE] = el[c]
        maps.append({"dlen": dpad.reshape(P, BLKS * DEG)})
    return maps


def _unshard(results):
    embs, cuts, keeps = [], [], []
    for r in results:
        emb = r["emb"].reshape(PAD_NODES, DEG, NB)[:NODES_PER_CORE]
        embs.append(emb.reshape(NODES_PER_CORE * DEG, NB))
        cut = r["cut"].reshape(PAD_NODES, DEG)[:NODES_PER_CORE]
        cuts.append(cut.reshape(-1))
        kp = r["keep"].reshape(PAD_NODES, DEG)[:NODES_PER_CORE]
        keeps.append(kp.reshape(-1))
    edge_emb = np.concatenate(embs, axis=0).astype(np.float32)
    cutoffs_edge = np.concatenate(cuts, axis=0).astype(np.float32)
    keep = np.concatenate(keeps, axis=0).astype(bool)
    return edge_emb, cutoffs_edge, keep


def run_on_hw(edge_length, **spmd_kwargs):
    nc = _get_nc()
    in_maps = _shard_inputs(edge_length)
    out = run_bass_kernel_spmd(nc, in_maps, list(range(N_CORES)), **spmd_kwargs)
    return out


def kernel(norm_length=None, edge_length=None, bessel_weights=None,
           edge_src=None, edge_dst=None, **_unused):
    """Full (unsharded) inputs -> full (edge_emb, cutoffs_edge, keep)."""
    assert edge_length is not None
    out = run_on_hw(edge_length)
    return _unshard(out.results)
